# revision 51
# baseline (speedup 1.0000x reference)
"""Trainium2 Bass kernel for AdaptivePersistenceLandscapeLayer.

Shards the batch (128 samples) across 8 NeuronCores (16 samples each).
Per core:
  - gathers birth/death filtration values from the dtm grid rows via
    gpsimd ap_gather on an fp16 copy of the rows (two calls cover the
    int16 index range), merged by a select
  - computes t_min/t_max per (sample, hom-dim) from the first <=2 pairs
    of that dim (cumsum ranks via tensor_tensor_scan)
  - evaluates tent functions on [t=128partition x 1024pair] tiles
    (4 t-chunks per (sample,dim)); top-1 via tensor_reduce, top-2 via
    exact argmax removal (is_equal + masked re-reduce); tiny per-pair
    perturbation makes tent values unique so duplicate-value argmax
    removal stays exact
  - clamps at 0, adds the adaptive time grid, writes [16, 2, 512, 2]
"""
import numpy as np

import concourse.bass as bass
import concourse.bacc as bacc
import concourse.mybir as mybir
from concourse.tile import TileContext

F32 = mybir.dt.float32
F16 = mybir.dt.float16
I32 = mybir.dt.int32
I16 = mybir.dt.int16
ALU = mybir.AluOpType
AX = mybir.AxisListType

B = 128
N = 65536
P = 1024
PW = 640               # per-dim padded pair-row width (max dim count ~512+8sigma)
T = 512
NCORES = 8
SPC = B // NCORES          # 16 samples per core
NSD = 2 * SPC              # 32 (sample, dim) rows
BIG = 1.0e30
EPS = 6.0e-8


def _install_axon_shim():
    import sys
    import types

    if "antenv.axon_hooks" in sys.modules:
        return
    mod = types.ModuleType("antenv.axon_hooks")
    mod._hook = None
    mod.set_axon_ntff_profile_hook = lambda h: setattr(mod, "_hook", h)
    mod.get_axon_ntff_profile_hook = lambda: mod._hook
    sys.modules["antenv.axon_hooks"] = mod
    try:
        import antenv

        antenv.axon_hooks = mod
    except ImportError:
        pass
    try:
        from trn_agent_boot.trn_boot import _ntff_profile_via_ctypes

        mod._hook = _ntff_profile_via_ctypes("/opt/axon/libaxon_pjrt.so")
    except Exception:
        pass


def build_nc():
    nc = bacc.Bacc("TRN2", target_bir_lowering=False, debug=False)
    dtm = nc.dram_tensor("dtm", [SPC, N], F32, kind="ExternalInput")
    locw = nc.dram_tensor("locw", [128, 320], I32, kind="ExternalInput")
    locc = nc.dram_tensor("locc", [40, 4 * PW], I32, kind="ExternalInput")
    pid_sd_in = nc.dram_tensor("pid_sd", [64, PW], F32, kind="ExternalInput")
    itc_in = nc.dram_tensor("itc", [128, 4], F32, kind="ExternalInput")
    ones_in = nc.dram_tensor("ones_c", [1, 128], F32, kind="ExternalInput")
    ident_in = nc.dram_tensor("ident", [128, 128], F32, kind="ExternalInput")
    out = nc.dram_tensor("out", [SPC, 2, T, 2], F32, kind="ExternalOutput")

    with TileContext(nc) as tc:
        with tc.tile_pool(name="gp0", bufs=1) as gp0, \
             tc.tile_pool(name="gp", bufs=1) as gp:
            # ---------------- gather phase ----------------
            bdp_lo = gp0.tile([40, 8 * PW], F16, tag="bdplo")
            bdp_hi = gp0.tile([40, 8 * PW], F16, tag="bdphi")
            locw_t = gp0.tile([128, 320], I32, tag="locw")
            nc.sync.dma_start(locw_t[:], locw[:])
            fli = gp0.tile([128, 320], I32, tag="fli")
            nc.vector.tensor_scalar(out=fli[:], in0=locw_t[:], scalar1=1,
                                    scalar2=None, op0=ALU.arith_shift_right)
            idxa_f = gp0.tile([128, 320], F32, tag="idxaf")
            nc.vector.tensor_copy(idxa_f[:], fli[:])
            idx_lo = gp0.tile([128, 320], I16, tag="idxlo")
            idx_hi = gp0.tile([128, 320], I16, tag="idxhi")
            tmpf = gp0.tile([128, 320], F32, tag="tmpf")
            tmpi = gp0.tile([128, 320], I32, tag="tmpi")
            # idx values are exact integers in f32; min/max keep them integral,
            # so the int cast is exact regardless of rounding mode
            nc.vector.tensor_scalar(out=tmpf[:], in0=idxa_f[:], scalar1=16383.0,
                                    scalar2=None, op0=ALU.min)
            nc.vector.tensor_copy(tmpi[:], tmpf[:])
            nc.vector.tensor_copy(idx_lo[:], tmpi[:])
            nc.vector.tensor_scalar(out=tmpf[:], in0=idxa_f[:], scalar1=-16384.0,
                                    scalar2=0.0, op0=ALU.add, op1=ALU.max)
            nc.vector.tensor_copy(tmpi[:], tmpf[:])
            nc.vector.tensor_copy(idx_hi[:], tmpi[:])

            with tc.tile_pool(name="dtp", bufs=1) as dtp:
                dt16 = dtp.tile([128, N // 2], F16, tag="dt16")
                ga = dtp.tile([128, 8 * PW], F16, tag="ga")
                nc.vector.memset(dt16[:], 0.0)
                _gsid, _ = nc.enter_named_scope("gatherphase", False)
                for rnd in range(2):
                    for h, (idx_t, bdp_t) in enumerate(
                            [(idx_lo, bdp_lo), (idx_hi, bdp_hi)]):
                        for c in range(8):
                            nc.gpsimd.dma_start(
                                dt16[16 * c:16 * c + 1, :],
                                dtm[8 * rnd + c:8 * rnd + c + 1,
                                    h * (N // 2):(h + 1) * (N // 2)])
                        nc.gpsimd.ap_gather(
                            ga[:], dt16[:], idx_t[:, 160 * rnd:160 * (rnd + 1)],
                            channels=128, num_elems=N // 4, d=2, num_idxs=4 * PW)
                        for c in range(8):
                            nc.sync.dma_start(
                                bdp_t[32 * rnd + c:32 * rnd + c + 1, :],
                                ga[16 * c:16 * c + 1, :])

            nc.leave_named_scope("gatherphase", _gsid, False)
            # pre-round select masks (rows 32r+si used; others ignored)
            locc_t = gp.tile([40, 4 * PW], I32, tag="locc")
            nc.sync.dma_start(locc_t[:], locc[:])
            pari = gp.tile([40, 4 * PW], I32, tag="pari")
            nc.vector.tensor_scalar(out=pari[:], in0=locc_t[:], scalar1=1,
                                    scalar2=None, op0=ALU.bitwise_and)
            par = gp.tile([40, 4 * PW], F32, tag="flh")
            nc.vector.tensor_copy(par[:], pari[:])
            nc.vector.tensor_scalar(out=pari[:], in0=locc_t[:], scalar1=32768,
                                    scalar2=None, op0=ALU.is_ge)
            hsel = gp.tile([40, 4 * PW], F32, tag="hsel")
            nc.vector.tensor_copy(hsel[:], pari[:])
            hm16 = gp.tile([40, 8 * PW], F16, tag="hm16")
            nc.vector.tensor_copy(
                hm16[:], hsel[:].unsqueeze(2).to_broadcast([40, 4 * PW, 2]))
            hm16n = gp.tile([40, 8 * PW], F16, tag="hm16n")
            nc.vector.tensor_scalar(out=hm16n[:], in0=hm16[:], scalar1=-1.0,
                                    scalar2=1.0, op0=ALU.mult, op1=ALU.add)
            parn = gp.tile([40, 4 * PW], F32, tag="hsel2")
            nc.vector.tensor_scalar(out=parn[:], in0=par[:], scalar1=-1.0,
                                    scalar2=1.0, op0=ALU.mult, op1=ALU.add)
            id_t = gp.tile([128, 128], F32, tag="ident")
            nc.sync.dma_start(id_t[:], ident_in[:])
            itc_t = gp.tile([128, 4], F32, tag="itc")
            nc.sync.dma_start(itc_t[:], itc_in[:])
            pid_sd = gp.tile([64, PW], F32, tag="pid_sd")
            nc.sync.dma_start(pid_sd[:], pid_sd_in[:])

            bdpm = gp.tile([40, 8 * PW], F16, tag="bdpm")
            bdf = gp.tile([40, 4 * PW], F32, tag="bdf")
            bdf2 = gp.tile([40, 4 * PW], F32, tag="pari")
            birth_sd = gp.tile([64, PW], F32, tag="birth_sd")
            death_sd = gp.tile([64, PW], F32, tag="death_sd")
            nb_sd = gp.tile([64, PW], F32, tag="nb_sd")
            tmin = gp.tile([64, 1], F32, tag="tmin")
            tmax = gp.tile([64, 1], F32, tag="tmax")
            delta = gp.tile([64, 1], F32, tag="delta")
            tm2 = gp.tile([64, 2], F32, tag="tm2")
            tseq = gp.tile([128, 64 * 4], F32, tag="tseq")
            lam = gp.tile([128, 64 * 4 * 2], F32, tag="lam")

            nc.vector.memset(lam[:], 0.0)
            nc.vector.memset(tseq[:], 0.0)
            _tsid, _ = nc.enter_named_scope("tentphase", False)
            with tc.tile_pool(name="wp", bufs=2) as wp, \
                 tc.tile_pool(name="pp0", bufs=1, space="PSUM") as pp0, \
                 tc.tile_pool(name="pp", bufs=1, space="PSUM") as pp:
                for rnd in range(2):
                    q = 32 * rnd
                    sl = slice(q, q + 8)          # merge rows
                    rl = slice(q, q + 16)         # sd rows
                    # merge this round's 8 samples
                    nc.vector.tensor_tensor(out=bdpm[sl, :], in0=bdp_hi[sl, :],
                                            in1=hm16[sl, :], op=ALU.mult)
                    nc.vector.tensor_tensor(out=hm16n[sl, :], in0=bdp_lo[sl, :],
                                            in1=hm16n[sl, :], op=ALU.mult)
                    nc.vector.tensor_tensor(out=bdpm[sl, :], in0=bdpm[sl, :],
                                            in1=hm16n[sl, :], op=ALU.add)
                    nc.vector.tensor_copy(bdf[sl, :], bdpm[sl, 0::2])
                    nc.vector.tensor_copy(bdf2[sl, :], bdpm[sl, 1::2])
                    nc.vector.tensor_tensor(out=bdf[sl, :], in0=bdf[sl, :],
                                            in1=parn[sl, :], op=ALU.mult)
                    nc.vector.tensor_tensor(out=bdf2[sl, :], in0=bdf2[sl, :],
                                            in1=par[sl, :], op=ALU.mult)
                    nc.vector.tensor_tensor(out=bdf[sl, :], in0=bdf[sl, :],
                                            in1=bdf2[sl, :], op=ALU.add)
                    # sd rows: q + d*8 + si  <- bdf rows q+si, col-block d
                    for d in range(2):
                        r0 = q + d * 8
                        nc.sync.dma_start(birth_sd[r0:r0 + 8, :],
                                          bdf[sl, d * PW:(d + 1) * PW])
                        nc.sync.dma_start(death_sd[r0:r0 + 8, :],
                                          bdf[sl, (2 + d) * PW:(3 + d) * PW])
                    nc.vector.tensor_reduce(out=tmin[rl, :],
                                            in_=birth_sd[rl, 0:2],
                                            axis=AX.X, op=ALU.min)
                    nc.vector.tensor_reduce(out=tmax[rl, :],
                                            in_=death_sd[rl, 0:2],
                                            axis=AX.X, op=ALU.max)
                    nc.vector.tensor_tensor(out=delta[rl, :], in0=tmax[rl, :],
                                            in1=tmin[rl, :], op=ALU.subtract)
                    nc.vector.tensor_scalar(out=delta[rl, :], in0=delta[rl, :],
                                            scalar1=1.0 / 511.0, scalar2=None,
                                            op0=ALU.mult)
                    nc.vector.tensor_copy(tm2[rl, 0:1], tmin[rl, :])
                    nc.vector.tensor_copy(tm2[rl, 1:2], delta[rl, :])
                    nc.vector.tensor_tensor(out=nb_sd[rl, :], in0=pid_sd[rl, :],
                                            in1=birth_sd[rl, :], op=ALU.subtract)
                    nc.vector.tensor_tensor(out=death_sd[rl, :],
                                            in0=death_sd[rl, :],
                                            in1=pid_sd[rl, :], op=ALU.add)
                    # tseq for this round's 16 sd rows
                    tm2T_p = pp0.tile([2, 16], F32, tag="tm2T")
                    nc.tensor.transpose(out=tm2T_p[:], in_=tm2[rl, :],
                                        identity=id_t[q:q + 16, q:q + 16])
                    tm2T = wp.tile([2, 16], F32, tag="tm2Ts")
                    nc.vector.tensor_copy(tm2T[:], tm2T_p[:])
                    tminb_p = pp0.tile([128, 16], F32, tag="tminb")
                    deltab_p = pp0.tile([128, 16], F32, tag="deltab")
                    nc.tensor.matmul(out=tminb_p[:],
                                     lhsT=id_t[:2, 0:1].to_broadcast([2, 128]),
                                     rhs=tm2T[:], start=True, stop=True)
                    nc.tensor.matmul(out=deltab_p[:],
                                     lhsT=id_t[:2, 1:2].to_broadcast([2, 128]),
                                     rhs=tm2T[:], start=True, stop=True)
                    itc_v = itc_t[:].unsqueeze(1).to_broadcast([128, 16, 4])
                    db_v = deltab_p[:].unsqueeze(2).to_broadcast([128, 16, 4])
                    tb_v = tminb_p[:].unsqueeze(2).to_broadcast([128, 16, 4])
                    tseq3 = tseq[:, q * 4:(q + 16) * 4].rearrange(
                        "p (a b) -> p a b", b=4)
                    nc.vector.tensor_tensor(out=tseq3, in0=itc_v, in1=db_v,
                                            op=ALU.mult)
                    nc.vector.tensor_tensor(out=tseq3, in0=tseq3, in1=tb_v,
                                            op=ALU.add)
                    # tents
                    for lrow in range(16):
                        row = q + lrow
                        nbb = pp.tile([128, PW], F32, tag="nbb")
                        dbb = pp.tile([128, PW], F32, tag="dbb")
                        sel_l = id_t[q:q + 16, row:row + 1].to_broadcast([16, 128])
                        for h0, h1 in ((0, 512), (512, PW)):
                            nc.tensor.matmul(
                                out=nbb[:, h0:h1], lhsT=sel_l,
                                rhs=nb_sd[rl, h0:h1], start=True, stop=True)
                            nc.tensor.matmul(
                                out=dbb[:, h0:h1], lhsT=sel_l,
                                rhs=death_sd[rl, h0:h1], start=True, stop=True)
                        for cp in range(2):
                            tent2 = wp.tile([128, 2 * PW], F32, tag="tent")
                            eq2 = wp.tile([128, 2 * PW], F32, tag="eq")
                            for cc in range(2):
                                c = 2 * cp + cc
                                tcol = tseq[:, row * 4 + c:row * 4 + c + 1]
                                ctv = eq2[:, cc * PW:(cc + 1) * PW]
                                nc.vector.tensor_scalar(
                                    out=ctv, in0=dbb[:], scalar1=tcol,
                                    scalar2=None, op0=ALU.subtract)
                                nc.vector.scalar_tensor_tensor(
                                    out=tent2[:, cc * PW:(cc + 1) * PW],
                                    in0=nbb[:], scalar=tcol, in1=ctv,
                                    op0=ALU.add, op1=ALU.min)
                            col0 = (row * 4 + 2 * cp) * 2
                            nc.vector.tensor_reduce(
                                out=lam[:, col0:col0 + 4:2],
                                in_=tent2[:].rearrange("p (a b) -> p a b", b=PW),
                                axis=AX.X, op=ALU.max)
                            for cc in range(2):
                                c = 2 * cp + cc
                                col = (row * 4 + c) * 2
                                nc.vector.tensor_scalar(
                                    out=eq2[:, cc * PW:(cc + 1) * PW],
                                    in0=tent2[:, cc * PW:(cc + 1) * PW],
                                    scalar1=lam[:, col:col + 1],
                                    scalar2=None, op0=ALU.is_equal)
                                nc.vector.scalar_tensor_tensor(
                                    out=eq2[:, cc * PW:(cc + 1) * PW],
                                    in0=eq2[:, cc * PW:(cc + 1) * PW], scalar=-BIG,
                                    in1=tent2[:, cc * PW:(cc + 1) * PW],
                                    op0=ALU.mult, op1=ALU.add)
                            nc.vector.tensor_reduce(
                                out=lam[:, col0 + 1:col0 + 5:2],
                                in_=eq2[:].rearrange("p (a b) -> p a b", b=PW),
                                axis=AX.X, op=ALU.max)
            nc.leave_named_scope("tentphase", _tsid, False)

            # clamp and add tseq
            nc.vector.tensor_scalar(out=lam[:], in0=lam[:], scalar1=0.0,
                                    scalar2=None, op0=ALU.max)
            tseq_r = tseq[:].unsqueeze(2).to_broadcast([128, 64 * 4, 2])
            lam3 = lam[:].rearrange("p (a b) -> p a b", b=2)
            nc.vector.tensor_tensor(out=lam3, in0=lam3, in1=tseq_r, op=ALU.add)

            # ---------------- output ----------------
            for rnd in range(2):
                for d in range(2):
                    for c in range(4):
                        dst = out.ap().rearrange("s d (c t) k -> s d c t k", c=4)[
                            8 * rnd:8 * rnd + 8, d, c, :, :].rearrange(
                            "s t k -> t s k")
                        src = lam[:].rearrange("t (b s c k) -> t b s c k",
                                               b=8, s=8, c=4)[:, 4 * rnd + d, :, c, :]
                        nc.sync.dma_start(dst, src)
    nc.compile()
    return nc


_NC_CACHE = None
_LAST_IN_MAPS = None


def kernel(dtm_val, birth_loc, death_loc, ph_dim):
    global _NC_CACHE
    _install_axon_shim()
    from concourse.bass_utils import run_bass_kernel_spmd

    dtm_val = np.ascontiguousarray(np.asarray(dtm_val, dtype=np.float32))
    birth_loc = np.asarray(birth_loc, dtype=np.int32)
    death_loc = np.asarray(death_loc, dtype=np.int32)
    ph_dim = np.asarray(ph_dim, dtype=np.int32)

    if _NC_CACHE is None:
        _NC_CACHE = build_nc()
    nc = _NC_CACHE

    itc = np.zeros((128, 4), np.float32)
    for c in range(4):
        itc[:, c] = 128 * c + np.arange(128)
    ones_c = np.ones((1, 128), np.float32)
    ident = np.eye(128, dtype=np.float32)

    in_maps = []
    for i in range(NCORES):
        s0 = i * SPC
        locc = np.zeros((40, 4 * PW), np.int32)
        pid_sd = np.full((64, PW), -BIG, np.float32)
        for si in range(SPC):
            rnd, sloc = si // 8, si % 8
            ph = ph_dim[s0 + si]
            for d in range(2):
                pos = np.where(ph == d)[0]
                assert len(pos) <= PW, f"dim count {len(pos)} exceeds PW={PW}"
                n = len(pos)
                locc[32 * rnd + sloc, d * PW:d * PW + n] = birth_loc[s0 + si, pos]
                locc[32 * rnd + sloc, (2 + d) * PW:(2 + d) * PW + n] = (
                    death_loc[s0 + si, pos])
                pid_sd[32 * rnd + d * 8 + sloc, :n] = pos.astype(np.float32) * EPS
        locw = np.zeros((128, 320), np.int32)
        for rnd in range(2):
            for c in range(8):
                lst = locc[32 * rnd + c]
                locw[16 * c:16 * (c + 1), 160 * rnd:160 * (rnd + 1)] = (
                    lst.reshape(160, 16).T)
        in_maps.append({
            "dtm": dtm_val[s0:s0 + SPC],
            "locw": locw,
            "locc": locc,
            "pid_sd": pid_sd,
            "itc": itc,
            "ones_c": ones_c,
            "ident": ident,
        })

    global _LAST_IN_MAPS
    _LAST_IN_MAPS = in_maps
    res = run_bass_kernel_spmd(nc, in_maps, core_ids=list(range(NCORES)))
    outs = [r["out"] for r in res.results]
    return np.concatenate(outs, axis=0).astype(np.float32)


# revision 53
# speedup vs baseline: 1.3028x; 1.3028x over previous
"""Trainium2 Bass kernel for AdaptivePersistenceLandscapeLayer.

Shards the batch (128 samples) across 8 NeuronCores (16 samples each).
Per core:
  - gathers birth/death filtration values from the dtm grid rows via
    gpsimd ap_gather on an fp16 copy of the rows (two calls cover the
    int16 index range), merged by a select
  - computes t_min/t_max per (sample, hom-dim) from the first <=2 pairs
    of that dim (cumsum ranks via tensor_tensor_scan)
  - evaluates tent functions on [t=128partition x 1024pair] tiles
    (4 t-chunks per (sample,dim)); top-1 via tensor_reduce, top-2 via
    exact argmax removal (is_equal + masked re-reduce); tiny per-pair
    perturbation makes tent values unique so duplicate-value argmax
    removal stays exact
  - clamps at 0, adds the adaptive time grid, writes [16, 2, 512, 2]
"""
import contextlib

import numpy as np

import concourse.bass as bass
import concourse.bacc as bacc
import concourse.mybir as mybir
from concourse.tile import TileContext

F32 = mybir.dt.float32
F16 = mybir.dt.float16
I32 = mybir.dt.int32
I16 = mybir.dt.int16
ALU = mybir.AluOpType
AX = mybir.AxisListType

B = 128
N = 65536
P = 1024
PW = 640               # per-dim padded pair-row width (max dim count ~512+8sigma)
T = 512
NCORES = 8
SPC = B // NCORES          # 16 samples per core
NSD = 2 * SPC              # 32 (sample, dim) rows
BIG = 1.0e30
EPS = 6.0e-8


def _install_axon_shim():
    import sys
    import types

    if "antenv.axon_hooks" in sys.modules:
        return
    mod = types.ModuleType("antenv.axon_hooks")
    mod._hook = None
    mod.set_axon_ntff_profile_hook = lambda h: setattr(mod, "_hook", h)
    mod.get_axon_ntff_profile_hook = lambda: mod._hook
    sys.modules["antenv.axon_hooks"] = mod
    try:
        import antenv

        antenv.axon_hooks = mod
    except ImportError:
        pass
    try:
        from trn_agent_boot.trn_boot import _ntff_profile_via_ctypes

        mod._hook = _ntff_profile_via_ctypes("/opt/axon/libaxon_pjrt.so")
    except Exception:
        pass


def build_nc():
    nc = bacc.Bacc("TRN2", target_bir_lowering=False, debug=False)
    dtm = nc.dram_tensor("dtm", [SPC, N], F32, kind="ExternalInput")
    locw = nc.dram_tensor("locw", [128, 320], I32, kind="ExternalInput")
    locc = nc.dram_tensor("locc", [40, 4 * PW], I32, kind="ExternalInput")
    pid_sd_in = nc.dram_tensor("pid_sd", [64, PW], F32, kind="ExternalInput")
    itc_in = nc.dram_tensor("itc", [128, 4], F32, kind="ExternalInput")
    ones_in = nc.dram_tensor("ones_c", [1, 128], F32, kind="ExternalInput")
    ident_in = nc.dram_tensor("ident", [128, 128], F32, kind="ExternalInput")
    out = nc.dram_tensor("out", [SPC, 2, T, 2], F32, kind="ExternalOutput")

    with TileContext(nc) as tc:
        with contextlib.ExitStack() as _st:
            gp0 = _st.enter_context(tc.tile_pool(name="gp0", bufs=1))
            # ---------------- gather phase ----------------
            # pair index loc>>1 spans [0, 32767] == int16 range: one full-row
            # fp16 table per 8 samples, single gather per round
            bdp = gp0.tile([40, 8 * PW], F16, tag="bdp")
            locw_t = gp0.tile([128, 320], I32, tag="locw")
            nc.sync.dma_start(locw_t[:], locw[:])
            fli = gp0.tile([128, 320], I32, tag="fli")
            nc.vector.tensor_scalar(out=fli[:], in0=locw_t[:], scalar1=1,
                                    scalar2=None, op0=ALU.arith_shift_right)
            idx16 = gp0.tile([128, 320], I16, tag="idx16")
            nc.vector.tensor_copy(idx16[:], fli[:])

            with tc.tile_pool(name="dtp", bufs=1) as dtp:
                dt16 = dtp.tile([128, N], F16, tag="dt16")
                ga = dtp.tile([128, 8 * PW], F16, tag="ga")
                nc.vector.memset(dt16[:, :N // 2], 0.0)
                nc.vector.memset(dt16[:, N // 2:], 0.0)
                _gsid, _ = nc.enter_named_scope("gatherphase", False)
                for rnd in range(2):
                    for c in range(8):
                        nc.gpsimd.dma_start(
                            dt16[16 * c:16 * c + 1, :],
                            dtm[8 * rnd + c:8 * rnd + c + 1, :])
                    nc.gpsimd.ap_gather(
                        ga[:], dt16[:], idx16[:, 160 * rnd:160 * (rnd + 1)],
                        channels=128, num_elems=N // 2, d=2, num_idxs=4 * PW)
                    for c in range(8):
                        nc.sync.dma_start(
                            bdp[32 * rnd + c:32 * rnd + c + 1, :],
                            ga[16 * c:16 * c + 1, :])

            nc.leave_named_scope("gatherphase", _gsid, False)
            gp = _st.enter_context(tc.tile_pool(name="gp", bufs=1))
            # pre-round select masks (rows 32r+si used; others ignored)
            locc_t = gp.tile([40, 4 * PW], I32, tag="locc")
            nc.sync.dma_start(locc_t[:], locc[:])
            pari = gp.tile([40, 4 * PW], I32, tag="pari")
            nc.vector.tensor_scalar(out=pari[:], in0=locc_t[:], scalar1=1,
                                    scalar2=None, op0=ALU.bitwise_and)
            par = gp.tile([40, 4 * PW], F32, tag="flh")
            nc.vector.tensor_copy(par[:], pari[:])
            parn = gp.tile([40, 4 * PW], F32, tag="hsel2")
            nc.vector.tensor_scalar(out=parn[:], in0=par[:], scalar1=-1.0,
                                    scalar2=1.0, op0=ALU.mult, op1=ALU.add)
            id_t = gp.tile([128, 128], F32, tag="ident")
            nc.sync.dma_start(id_t[:], ident_in[:])
            itc_t = gp.tile([128, 4], F32, tag="itc")
            nc.sync.dma_start(itc_t[:], itc_in[:])
            pid_sd = gp.tile([64, PW], F32, tag="pid_sd")
            nc.sync.dma_start(pid_sd[:], pid_sd_in[:])

            bdf = gp.tile([40, 4 * PW], F32, tag="bdf")
            bdf2 = gp.tile([40, 4 * PW], F32, tag="pari")
            birth_sd = gp.tile([64, PW], F32, tag="birth_sd")
            death_sd = gp.tile([64, PW], F32, tag="death_sd")
            nb_sd = gp.tile([64, PW], F32, tag="nb_sd")
            tmin = gp.tile([64, 1], F32, tag="tmin")
            tmax = gp.tile([64, 1], F32, tag="tmax")
            delta = gp.tile([64, 1], F32, tag="delta")
            tm2 = gp.tile([64, 2], F32, tag="tm2")
            tseq = gp.tile([128, 64 * 4], F32, tag="tseq")
            lam = gp.tile([128, 64 * 4 * 2], F32, tag="lam")

            nc.vector.memset(lam[:], 0.0)
            nc.vector.memset(tseq[:], 0.0)
            _tsid, _ = nc.enter_named_scope("tentphase", False)
            with tc.tile_pool(name="wp", bufs=2) as wp, \
                 tc.tile_pool(name="pp0", bufs=1, space="PSUM") as pp0, \
                 tc.tile_pool(name="pp", bufs=1, space="PSUM") as pp:
                for rnd in range(2):
                    q = 32 * rnd
                    sl = slice(q, q + 8)          # merge rows
                    rl = slice(q, q + 16)         # sd rows
                    # merge this round's 8 samples (parity select within pair)
                    nc.vector.tensor_copy(bdf[sl, :], bdp[sl, 0::2])
                    nc.vector.tensor_copy(bdf2[sl, :], bdp[sl, 1::2])
                    nc.vector.tensor_tensor(out=bdf[sl, :], in0=bdf[sl, :],
                                            in1=parn[sl, :], op=ALU.mult)
                    nc.vector.tensor_tensor(out=bdf2[sl, :], in0=bdf2[sl, :],
                                            in1=par[sl, :], op=ALU.mult)
                    nc.vector.tensor_tensor(out=bdf[sl, :], in0=bdf[sl, :],
                                            in1=bdf2[sl, :], op=ALU.add)
                    # sd rows: q + d*8 + si  <- bdf rows q+si, col-block d
                    for d in range(2):
                        r0 = q + d * 8
                        nc.sync.dma_start(birth_sd[r0:r0 + 8, :],
                                          bdf[sl, d * PW:(d + 1) * PW])
                        nc.sync.dma_start(death_sd[r0:r0 + 8, :],
                                          bdf[sl, (2 + d) * PW:(3 + d) * PW])
                    nc.vector.tensor_reduce(out=tmin[rl, :],
                                            in_=birth_sd[rl, 0:2],
                                            axis=AX.X, op=ALU.min)
                    nc.vector.tensor_reduce(out=tmax[rl, :],
                                            in_=death_sd[rl, 0:2],
                                            axis=AX.X, op=ALU.max)
                    nc.vector.tensor_tensor(out=delta[rl, :], in0=tmax[rl, :],
                                            in1=tmin[rl, :], op=ALU.subtract)
                    nc.vector.tensor_scalar(out=delta[rl, :], in0=delta[rl, :],
                                            scalar1=1.0 / 511.0, scalar2=None,
                                            op0=ALU.mult)
                    nc.vector.tensor_copy(tm2[rl, 0:1], tmin[rl, :])
                    nc.vector.tensor_copy(tm2[rl, 1:2], delta[rl, :])
                    nc.vector.tensor_tensor(out=nb_sd[rl, :], in0=pid_sd[rl, :],
                                            in1=birth_sd[rl, :], op=ALU.subtract)
                    nc.vector.tensor_tensor(out=death_sd[rl, :],
                                            in0=death_sd[rl, :],
                                            in1=pid_sd[rl, :], op=ALU.add)
                    # tseq for this round's 16 sd rows
                    tm2T_p = pp0.tile([2, 16], F32, tag="tm2T")
                    nc.tensor.transpose(out=tm2T_p[:], in_=tm2[rl, :],
                                        identity=id_t[q:q + 16, q:q + 16])
                    tm2T = wp.tile([2, 16], F32, tag="tm2Ts")
                    nc.vector.tensor_copy(tm2T[:], tm2T_p[:])
                    tminb_p = pp0.tile([128, 16], F32, tag="tminb")
                    deltab_p = pp0.tile([128, 16], F32, tag="deltab")
                    nc.tensor.matmul(out=tminb_p[:],
                                     lhsT=id_t[:2, 0:1].to_broadcast([2, 128]),
                                     rhs=tm2T[:], start=True, stop=True)
                    nc.tensor.matmul(out=deltab_p[:],
                                     lhsT=id_t[:2, 1:2].to_broadcast([2, 128]),
                                     rhs=tm2T[:], start=True, stop=True)
                    itc_v = itc_t[:].unsqueeze(1).to_broadcast([128, 16, 4])
                    db_v = deltab_p[:].unsqueeze(2).to_broadcast([128, 16, 4])
                    tb_v = tminb_p[:].unsqueeze(2).to_broadcast([128, 16, 4])
                    tseq3 = tseq[:, q * 4:(q + 16) * 4].rearrange(
                        "p (a b) -> p a b", b=4)
                    nc.vector.tensor_tensor(out=tseq3, in0=itc_v, in1=db_v,
                                            op=ALU.mult)
                    nc.vector.tensor_tensor(out=tseq3, in0=tseq3, in1=tb_v,
                                            op=ALU.add)
                    # tents
                    for lrow in range(16):
                        row = q + lrow
                        nbb = pp.tile([128, PW], F32, tag="nbb")
                        dbb = pp.tile([128, PW], F32, tag="dbb")
                        sel_l = id_t[q:q + 16, row:row + 1].to_broadcast([16, 128])
                        for h0, h1 in ((0, 512), (512, PW)):
                            nc.tensor.matmul(
                                out=nbb[:, h0:h1], lhsT=sel_l,
                                rhs=nb_sd[rl, h0:h1], start=True, stop=True)
                            nc.tensor.matmul(
                                out=dbb[:, h0:h1], lhsT=sel_l,
                                rhs=death_sd[rl, h0:h1], start=True, stop=True)
                        for cp in range(2):
                            tent2 = wp.tile([128, 2 * PW], F32, tag="tent")
                            eq2 = wp.tile([128, 2 * PW], F32, tag="eq")
                            for cc in range(2):
                                c = 2 * cp + cc
                                tcol = tseq[:, row * 4 + c:row * 4 + c + 1]
                                ctv = eq2[:, cc * PW:(cc + 1) * PW]
                                nc.vector.tensor_scalar(
                                    out=ctv, in0=dbb[:], scalar1=tcol,
                                    scalar2=None, op0=ALU.subtract)
                                nc.vector.scalar_tensor_tensor(
                                    out=tent2[:, cc * PW:(cc + 1) * PW],
                                    in0=nbb[:], scalar=tcol, in1=ctv,
                                    op0=ALU.add, op1=ALU.min)
                            col0 = (row * 4 + 2 * cp) * 2
                            nc.vector.tensor_reduce(
                                out=lam[:, col0:col0 + 4:2],
                                in_=tent2[:].rearrange("p (a b) -> p a b", b=PW),
                                axis=AX.X, op=ALU.max)
                            for cc in range(2):
                                c = 2 * cp + cc
                                col = (row * 4 + c) * 2
                                nc.vector.tensor_scalar(
                                    out=eq2[:, cc * PW:(cc + 1) * PW],
                                    in0=tent2[:, cc * PW:(cc + 1) * PW],
                                    scalar1=lam[:, col:col + 1],
                                    scalar2=None, op0=ALU.is_equal)
                                nc.vector.scalar_tensor_tensor(
                                    out=eq2[:, cc * PW:(cc + 1) * PW],
                                    in0=eq2[:, cc * PW:(cc + 1) * PW], scalar=-BIG,
                                    in1=tent2[:, cc * PW:(cc + 1) * PW],
                                    op0=ALU.mult, op1=ALU.add)
                            nc.vector.tensor_reduce(
                                out=lam[:, col0 + 1:col0 + 5:2],
                                in_=eq2[:].rearrange("p (a b) -> p a b", b=PW),
                                axis=AX.X, op=ALU.max)
            nc.leave_named_scope("tentphase", _tsid, False)

            # clamp and add tseq
            nc.vector.tensor_scalar(out=lam[:], in0=lam[:], scalar1=0.0,
                                    scalar2=None, op0=ALU.max)
            tseq_r = tseq[:].unsqueeze(2).to_broadcast([128, 64 * 4, 2])
            lam3 = lam[:].rearrange("p (a b) -> p a b", b=2)
            nc.vector.tensor_tensor(out=lam3, in0=lam3, in1=tseq_r, op=ALU.add)

            # ---------------- output ----------------
            for rnd in range(2):
                for d in range(2):
                    for c in range(4):
                        dst = out.ap().rearrange("s d (c t) k -> s d c t k", c=4)[
                            8 * rnd:8 * rnd + 8, d, c, :, :].rearrange(
                            "s t k -> t s k")
                        src = lam[:].rearrange("t (b s c k) -> t b s c k",
                                               b=8, s=8, c=4)[:, 4 * rnd + d, :, c, :]
                        nc.sync.dma_start(dst, src)
    nc.compile()
    return nc


_NC_CACHE = None
_LAST_IN_MAPS = None


def kernel(dtm_val, birth_loc, death_loc, ph_dim):
    global _NC_CACHE
    _install_axon_shim()
    from concourse.bass_utils import run_bass_kernel_spmd

    dtm_val = np.ascontiguousarray(np.asarray(dtm_val, dtype=np.float32))
    birth_loc = np.asarray(birth_loc, dtype=np.int32)
    death_loc = np.asarray(death_loc, dtype=np.int32)
    ph_dim = np.asarray(ph_dim, dtype=np.int32)

    if _NC_CACHE is None:
        _NC_CACHE = build_nc()
    nc = _NC_CACHE

    itc = np.zeros((128, 4), np.float32)
    for c in range(4):
        itc[:, c] = 128 * c + np.arange(128)
    ones_c = np.ones((1, 128), np.float32)
    ident = np.eye(128, dtype=np.float32)

    in_maps = []
    for i in range(NCORES):
        s0 = i * SPC
        locc = np.zeros((40, 4 * PW), np.int32)
        pid_sd = np.full((64, PW), -BIG, np.float32)
        for si in range(SPC):
            rnd, sloc = si // 8, si % 8
            ph = ph_dim[s0 + si]
            for d in range(2):
                pos = np.where(ph == d)[0]
                assert len(pos) <= PW, f"dim count {len(pos)} exceeds PW={PW}"
                n = len(pos)
                locc[32 * rnd + sloc, d * PW:d * PW + n] = birth_loc[s0 + si, pos]
                locc[32 * rnd + sloc, (2 + d) * PW:(2 + d) * PW + n] = (
                    death_loc[s0 + si, pos])
                pid_sd[32 * rnd + d * 8 + sloc, :n] = pos.astype(np.float32) * EPS
        locw = np.zeros((128, 320), np.int32)
        for rnd in range(2):
            for c in range(8):
                lst = locc[32 * rnd + c]
                locw[16 * c:16 * (c + 1), 160 * rnd:160 * (rnd + 1)] = (
                    lst.reshape(160, 16).T)
        in_maps.append({
            "dtm": dtm_val[s0:s0 + SPC],
            "locw": locw,
            "locc": locc,
            "pid_sd": pid_sd,
            "itc": itc,
            "ones_c": ones_c,
            "ident": ident,
        })

    global _LAST_IN_MAPS
    _LAST_IN_MAPS = in_maps
    res = run_bass_kernel_spmd(nc, in_maps, core_ids=list(range(NCORES)))
    outs = [r["out"] for r in res.results]
    return np.concatenate(outs, axis=0).astype(np.float32)
